# revision 9
# baseline (speedup 1.0000x reference)
"""Trainium2 Bass kernel for a 2-layer edge-gated GCN (DiffGNNPlacement).

Math (reference, per layer):
    ew   = 0.5 + sigmoid(edge_logits)                  # [E]
    deg  = segsum(ew -> col) + 1                       # [N]
    dis  = deg^-1/2
    norm = dis[row] * ew * dis[col]                    # [E]
    out  = segsum(norm * (h@W)[row] -> col) + (h@W)*dis^2 + b

Key transform: aggregation commutes with the (linear) feature transform, so
    out = (segsum(norm * h[row] -> col) + h*dis^2) @ W + b
and the self-loop term (h*dis^2, transposed) is added host-side via sxT.

Device algorithm (per core, nodes sharded 12500/core):
  - edges partitioned by target shard, sorted by target col, grouped by
    source chunk of 25000 rows (dma_gather indices are int16), packed into
    128-slot tiles spanning <=32 target cols within a 512-col PSUM window.
  - per tile: dma_gather 128 rows (256B each: 64 bf16 channels + pad) of the
    feature table -> SBUF; a host-built one-hot-times-norm matrix S
    [128, 32] bf16; PE matmul psum[64, off:off+w] += gathered^T @ S.
  - the 4 source groups use the 4 SWDGE queues (queue_num=g), so descriptor
    generation runs on all 8 GPSIMD Q7 cores concurrently (4 pairs) instead
    of serializing on pair 0 -- the single biggest win over one queue.
  - per window: psum + sxT window -> z window (f32), then immediately the
    dense stage for that window (phase A: h = relu(z@W1+b1) row-major, bf16,
    padded rows for the next layer's gather table; phase B: transposed
    dense + classifier head -> outT), overlapping the tail with later
    windows' gathers.

Two specialized programs per core (A: agg+dense, B: agg+head); host
all-gathers h1 between the launches.
"""

import os
import sys
import numpy as np
from contextlib import ExitStack

for _p in ("/opt/trn_rl_repo", "/root/.axon_site/_ro/trn_rl_repo"):
    if os.path.isdir(_p) and _p not in sys.path:
        sys.path.insert(0, _p)

import ml_dtypes

BF16 = np.dtype(ml_dtypes.bfloat16)


# ----------------------------------------------------------------- config ---
class Cfg:
    def __init__(self, N=100000, E=1600000, C=64, H2=32, P=8,
                 SRC_CHUNK=25000, W=32, WIN=512, TCH=32):
        self.N, self.E, self.C, self.H2, self.P = N, E, C, H2, P
        self.NLOC = N // P
        self.SRC_CHUNK = SRC_CHUNK
        self.NGRP = (N + SRC_CHUNK - 1) // SRC_CHUNK
        self.W = W            # S tile width (target-col window per tile)
        self.WIN = WIN        # PSUM accumulation window (cols)
        self.TCH = TCH        # tiles per gather chunk
        self.NWIN = (self.NLOC + WIN - 1) // WIN
        self.ROWB = 128       # padded table row: 128 bf16 elems = 256B
        assert SRC_CHUNK <= 32767
        assert self.NGRP == 4  # one SWDGE queue per source group


FULL = Cfg()


# --------------------------------------------------------- host preprocess ---
def _sigmoid(x):
    return 0.5 * (np.tanh(0.5 * x) + 1.0)


def preprocess(edge_index, edge_logits, cfg=FULL):
    """Compute norms and per-device tile plans (pure numpy)."""
    N, NLOC, G = cfg.N, cfg.NLOC, cfg.NGRP
    row = np.asarray(edge_index[0], dtype=np.int64)
    col = np.asarray(edge_index[1], dtype=np.int64)
    ew = (0.5 + _sigmoid(np.asarray(edge_logits, dtype=np.float32))).astype(np.float32)
    deg = np.bincount(col, weights=ew.astype(np.float64), minlength=N).astype(np.float32) + 1.0
    dis = deg ** -0.5
    norm = (dis[row] * ew * dis[col]).astype(np.float32)

    a_row, a_col, a_val = row, col, norm
    dev = a_col // NLOC
    grp = a_row // cfg.SRC_CHUNK
    order = np.lexsort((a_col, grp, dev))
    a_row, a_col, a_val = a_row[order], a_col[order], a_val[order]
    dev, grp = dev[order], grp[order]

    key = dev * G + grp
    bounds = np.searchsorted(key, np.arange(cfg.P * G + 1))
    plans = []
    for d in range(cfg.P):
        gplans = []
        for g in range(G):
            a, b = bounds[d * G + g], bounds[d * G + g + 1]
            gplans.append(_plan_group(
                (a_row[a:b] - g * cfg.SRC_CHUNK).astype(np.int16),
                (a_col[a:b] - d * NLOC).astype(np.int32),
                a_val[a:b], cfg))
        plans.append(gplans)
    return plans, dis


def _plan_group(rows, cols, vals, cfg):
    """Tile a sorted-by-col edge list: 128-slot tiles, <=W col span (tiles
    may straddle a WIN boundary). Returns packed gather/S arrays."""
    m = len(cols)
    starts, c0s = [], []
    i = 0
    while i < m:
        c0 = int(cols[i])
        # span-limited only; tiles may straddle a WIN boundary (the matmul
        # is split into two PSUM windows at build time)
        jmax = min(i + 128, m)
        j = i + int(np.searchsorted(cols[i:jmax], c0 + cfg.W, side="left"))
        starts.append(i)
        c0s.append(c0)
        i = j
    T = len(c0s)
    starts_a = np.array(starts + [m], dtype=np.int64)
    c0s = np.array(c0s, dtype=np.int32)

    tile_of = np.repeat(np.arange(T), np.diff(starts_a))
    slot = np.arange(m) - starts_a[tile_of]
    idx16 = np.zeros((T, 128), np.int16)
    idx16[tile_of, slot] = rows
    S = np.zeros((T, 128, cfg.W), np.float32)
    S[tile_of, slot, cols - c0s[tile_of]] = vals

    # chunk packing
    TCH = cfg.TCH
    nch = max(1, (T + TCH - 1) // TCH)
    Tp = nch * TCH
    flat = np.zeros(Tp * 128, np.int16)
    flat[: T * 128] = idx16.reshape(-1)
    # wrap: idx i -> [i % 16, i // 16], replicated across 8 groups of 16 partitions
    wrapped = flat.reshape(nch, TCH * 128 // 16, 16).transpose(0, 2, 1)  # [nch,16,TCH*8]
    idx_w = np.ascontiguousarray(np.tile(wrapped, (1, 8, 1)))            # [nch,128,TCH*8]
    Sp = np.zeros((Tp, 128, cfg.W), np.float32)
    Sp[:T] = S
    S_pk = np.ascontiguousarray(
        Sp.reshape(nch, TCH, 128, cfg.W).transpose(0, 2, 1, 3)).astype(BF16)  # [nch,128,TCH,W]
    nids = [min(TCH, T - ch * TCH) * 128 for ch in range(nch)]

    win = c0s // cfg.WIN
    off = c0s - win * cfg.WIN
    return dict(T=T, nch=nch, idx=idx_w, S=S_pk, nids=nids, win=win, off=off)


# ---------------------------------------------------------- program builder ---
def build_program(plan_d, phase, cfg=FULL, name="gnn"):
    import concourse.bass as bass
    import concourse.mybir as mybir
    from concourse import bacc
    from concourse.tile import TileContext

    f32, bf16, i16 = mybir.dt.float32, mybir.dt.bfloat16, mybir.dt.int16
    C, W, WIN, TCH, NLOC = cfg.C, cfg.W, cfg.WIN, cfg.TCH, cfg.NLOC
    G, ROWB = cfg.NGRP, cfg.ROWB

    nc = bacc.Bacc("TRN2", enable_partition_id=False,
                   target_bir_lowering=False, name=name,
                   num_swdge_queues=4)

    table = nc.dram_tensor("table", [cfg.N, ROWB], bf16, kind="ExternalInput")
    sxT_dr = nc.dram_tensor("sxT", [C, NLOC], f32, kind="ExternalInput")
    Wd = nc.dram_tensor("Wd", [C, C], f32, kind="ExternalInput")
    bb_dr = nc.dram_tensor("bb", [128, C], f32, kind="ExternalInput")
    bdc = nc.dram_tensor("bdc", [C, 1], f32, kind="ExternalInput")
    lw = nc.dram_tensor("lw", [C, 1], f32, kind="ExternalInput")
    lb = nc.dram_tensor("lb", [1, 1], f32, kind="ExternalInput")
    idx_dr, S_dr = [], []
    for g in range(G):
        p = plan_d[g]
        idx_dr.append(nc.dram_tensor(f"idx{g}", list(p["idx"].shape), i16,
                                     kind="ExternalInput"))
        S_dr.append(nc.dram_tensor(f"S{g}", list(p["S"].shape), bf16,
                                   kind="ExternalInput"))
    if phase == "A":
        h_out = nc.dram_tensor("h_out", [NLOC, ROWB], bf16, kind="ExternalOutput")
    else:
        outT = nc.dram_tensor("outT", [2, NLOC], f32, kind="ExternalOutput")

    # per-window tile lists: (g, t, off, s_lo, weff); straddling tiles get
    # a second entry in the next window with an S-column offset
    win_tiles = [[] for _ in range(cfg.NWIN)]
    for g in range(G):
        p = plan_d[g]
        for t in range(p["T"]):
            w = int(p["win"][t])
            off = int(p["off"][t])
            wlen = min(WIN, NLOC - w * WIN)
            w1 = min(W, WIN - off)
            win_tiles[w].append((g, t, off, 0, min(w1, wlen - off)))
            if W > w1 and w + 1 < cfg.NWIN:
                wlen2 = min(WIN, NLOC - (w + 1) * WIN)
                win_tiles[w + 1].append((g, t, 0, w1, min(W - w1, wlen2)))

    with TileContext(nc) as tc, ExitStack() as ex:
        cpool = ex.enter_context(tc.tile_pool(name="consts", bufs=1))
        zpool = ex.enter_context(tc.tile_pool(name="z", bufs=3))
        sxpool = ex.enter_context(tc.tile_pool(name="sx", bufs=3))
        gpools = [ex.enter_context(tc.tile_pool(name=f"gat{g}", bufs=3)) for g in range(G)]
        ipools = [ex.enter_context(tc.tile_pool(name=f"idx{g}", bufs=4)) for g in range(G)]
        spools = [ex.enter_context(tc.tile_pool(name=f"s{g}", bufs=3)) for g in range(G)]
        ppool = ex.enter_context(tc.tile_pool(name="psagg", bufs=2, space="PSUM"))
        if phase == "A":
            pdpool = ex.enter_context(tc.tile_pool(name="psd", bufs=3, space="PSUM"))
            hpool = ex.enter_context(tc.tile_pool(name="hrows", bufs=3))
        else:
            ptpool = ex.enter_context(tc.tile_pool(name="pst", bufs=2, space="PSUM"))
            plpool = ex.enter_context(tc.tile_pool(name="psl", bufs=2, space="PSUM"))
            htpool = ex.enter_context(tc.tile_pool(name="ht", bufs=2))
            opool = ex.enter_context(tc.tile_pool(name="ot", bufs=3))

        # ---- constants
        zrow = cpool.tile([1, WIN], bf16)
        nc.vector.memset(zrow[:, :], 0.0)
        Wd_sb = cpool.tile([C, C], f32)
        nc.sync.dma_start(out=Wd_sb[:, :], in_=Wd[:, :])
        if phase == "A":
            bb = cpool.tile([128, C], f32)
            nc.sync.dma_start(out=bb[:, :], in_=bb_dr[:, :])
        else:
            bd_col = cpool.tile([C, 1], f32)
            nc.sync.dma_start(out=bd_col[:, :], in_=bdc[:, :])
            lw_sb = cpool.tile([C, 1], f32)
            nc.sync.dma_start(out=lw_sb[:, :], in_=lw[:, :])
            lb_sb = cpool.tile([1, 1], f32)
            nc.sync.dma_start(out=lb_sb[:, :], in_=lb[:, :])
            nlb = cpool.tile([1, 1], f32)
            nc.scalar.mul(nlb[:, :], lb_sb[:, :], -1.0)

        # ---- aggregation state
        cur = [dict(ch=-1, gb=None, sb=None) for _ in range(G)]

        def ensure_chunk(g, ch):
            st = cur[g]
            if st["ch"] == ch:
                return st
            p = plan_d[g]
            ntl = min(TCH, p["T"] - ch * TCH)
            nid = p["nids"][ch]
            ib = ipools[g].tile([128, TCH * 8], i16, tag="idx")
            nc.sync.dma_start(out=ib[:, : ntl * 8], in_=idx_dr[g][ch, :, : ntl * 8])
            sb = spools[g].tile([128, TCH, W], bf16, tag="s")
            nc.scalar.dma_start(out=sb[:, :ntl, :], in_=S_dr[g][ch, :, :ntl, :])
            gb = gpools[g].tile([128, TCH, ROWB], bf16, tag="g")
            # last chunk: split into sub-gathers so the final windows' matmuls
            # unblock progressively instead of waiting for one 32-tile gather
            subs = [(0, ntl)] if ch < p["nch"] - 1 else \
                [(a, min(a + 8, ntl)) for a in range(0, ntl, 8)]
            for lo, hi in subs:
                nc.gpsimd.dma_gather(
                    gb[:, lo:hi, :],
                    table[g * cfg.SRC_CHUNK:(g + 1) * cfg.SRC_CHUNK, :],
                    ib[:, lo * 8: hi * 8],
                    (hi - lo) * 128, (hi - lo) * 128, ROWB,
                    single_packet=False,
                    queue_num=g,
                )
            st.update(ch=ch, gb=gb, sb=sb)
            return st

        # prefetch the first chunks so gathers start before the consts DMAs
        for g in range(G):
            ensure_chunk(g, 0)

        for w in range(cfg.NWIN):
            wlen = min(WIN, NLOC - w * WIN)
            ps = ppool.tile([C, WIN], f32)
            nc.tensor.matmul(ps[:, :wlen], lhsT=zrow[:, :C], rhs=zrow[:, :wlen],
                             start=True, stop=False)
            for g, t, off, s_lo, weff in win_tiles[w]:
                st = ensure_chunk(g, t // TCH)
                tp = t % TCH
                nc.tensor.matmul(
                    ps[:, off:off + weff],
                    lhsT=st["gb"][:, tp, :C],
                    rhs=st["sb"][:, tp, s_lo:s_lo + weff],
                    start=False, stop=False,
                    skip_group_check=True,
                )
            nc.tensor.matmul(ps[:, :wlen], lhsT=zrow[:, :C], rhs=zrow[:, :wlen],
                             start=False, stop=True)
            sxw = sxpool.tile([C, WIN], f32, tag="sx")
            nc.sync.dma_start(out=sxw[:, :wlen],
                              in_=sxT_dr[:, w * WIN:w * WIN + wlen])
            zw = zpool.tile([C, WIN], f32, tag="z")
            nc.vector.tensor_tensor(out=zw[:, :wlen], in0=ps[:, :wlen],
                                    in1=sxw[:, :wlen], op=mybir.AluOpType.add)

            if phase == "A":
                # dense, row-major (next layer's gather table), this window
                r0 = w * WIN
                nck = (wlen + 127) // 128
                hb = hpool.tile([128, (WIN + 127) // 128, C], bf16, tag="h")
                for kk in range(nck):
                    mrow = min(128, wlen - kk * 128)
                    psd = pdpool.tile([128, C], f32)
                    nc.tensor.matmul(psd[:mrow, :],
                                     lhsT=zw[:, kk * 128:kk * 128 + mrow],
                                     rhs=Wd_sb[:, :], start=True, stop=True)
                    nc.vector.tensor_tensor(out=hb[:mrow, kk, :], in0=psd[:mrow, :],
                                            in1=bb[:mrow, :], op=mybir.AluOpType.add)
                    nc.scalar.activation(hb[:mrow, kk, :], hb[:mrow, kk, :],
                                         mybir.ActivationFunctionType.Relu)
                nfull = wlen // 128
                if nfull:
                    dst = h_out[r0:r0 + nfull * 128, :C].rearrange(
                        "(t p) c -> p t c", p=128)
                    nc.sync.dma_start(out=dst, in_=hb[:, :nfull, :])
                rem = wlen - nfull * 128
                if rem:
                    nc.sync.dma_start(out=h_out[r0 + nfull * 128:r0 + wlen, :C],
                                      in_=hb[:rem, nfull, :])
            else:
                # transposed dense + classifier head, this window
                pst = ptpool.tile([C, WIN], f32)
                nc.tensor.matmul(pst[:, :wlen], lhsT=Wd_sb[:, :],
                                 rhs=zw[:, :wlen], start=True, stop=True)
                ht = htpool.tile([C, WIN], f32, tag="ht")
                nc.scalar.activation(ht[:, :wlen], pst[:, :wlen],
                                     mybir.ActivationFunctionType.Relu,
                                     bias=bd_col[:, :])
                psl = plpool.tile([1, WIN], f32)
                nc.tensor.matmul(psl[:, :wlen], lhsT=lw_sb[:, :], rhs=ht[:, :wlen],
                                 start=True, stop=True)
                otn = opool.tile([1, WIN], f32, tag="otn")
                otp = opool.tile([1, WIN], f32, tag="otp")
                nc.scalar.activation(otn[:, :wlen], psl[:, :wlen],
                                     mybir.ActivationFunctionType.Identity,
                                     bias=nlb[:, :], scale=-1.0)
                nc.scalar.activation(otp[:, :wlen], psl[:, :wlen],
                                     mybir.ActivationFunctionType.Identity,
                                     bias=lb_sb[:, :], scale=1.0)
                nc.sync.dma_start(out=outT[0:1, w * WIN:w * WIN + wlen],
                                  in_=otn[:, :wlen])
                nc.sync.dma_start(out=outT[1:2, w * WIN:w * WIN + wlen],
                                  in_=otp[:, :wlen])

    nc.compile()
    return nc


# ------------------------------------------------------------------ runner ---
def make_runner(nc, device):
    """Single-core jit runner pinned to one device, reusable across calls."""
    import jax
    import concourse.mybir as mybir
    from concourse import bass2jax

    bass2jax.install_neuronx_cc_hook()

    in_names, out_names, out_avals, zero_shapes = [], [], [], []
    for alloc in nc.m.functions[0].allocations:
        if not isinstance(alloc, mybir.MemoryLocationSet):
            continue
        nm = alloc.memorylocations[0].name
        if alloc.kind == "ExternalInput":
            in_names.append(nm)
        elif alloc.kind == "ExternalOutput":
            shape = tuple(alloc.tensor_shape)
            dtype = mybir.dt.np(alloc.dtype)
            out_names.append(nm)
            out_avals.append(jax.core.ShapedArray(shape, dtype))
            zero_shapes.append((shape, dtype))
    n_params = len(in_names)
    all_in_names = in_names + out_names
    donate = tuple(range(n_params, n_params + len(out_names)))

    def _body(*args):
        outs = bass2jax._bass_exec_p.bind(
            *args,
            out_avals=tuple(out_avals),
            in_names=tuple(all_in_names),
            out_names=tuple(out_names),
            lowering_input_output_aliases=(),
            sim_require_finite=True,
            sim_require_nnan=True,
            nc=nc,
        )
        return tuple(outs)

    jitted = jax.jit(_body, donate_argnums=donate, keep_unused=True)

    def run(in_map):
        args = [jax.device_put(np.asarray(in_map[nm]), device) for nm in in_names]
        zeros = [jax.device_put(np.zeros(s, d), device) for s, d in zero_shapes]
        outs = jitted(*args, *zeros)
        return {nm: outs[i] for i, nm in enumerate(out_names)}

    return run


# ---------------------------------------------------------------- kernel() ---
_CACHE = {}


def _get_runners(plans, cfg):
    import jax
    from concurrent.futures import ThreadPoolExecutor
    key = "runners"
    if key in _CACHE:
        return _CACHE[key]
    devices = jax.devices()[:cfg.P]

    def build_pair(d):
        ncA = build_program(plans[d], "A", cfg, name=f"gnnA_d{d}")
        ncB = build_program(plans[d], "B", cfg, name=f"gnnB_d{d}")
        return (make_runner(ncA, devices[d]), make_runner(ncB, devices[d]))

    with ThreadPoolExecutor(4) as exe:
        runners = list(exe.map(build_pair, range(cfg.P)))
    _CACHE[key] = runners
    return runners


def run_two_phase(inputs, cfg=FULL):
    from concurrent.futures import ThreadPoolExecutor

    x = np.asarray(inputs["x"], np.float32)
    W1 = np.asarray(inputs["W1"], np.float32)
    b1 = np.asarray(inputs["b1"], np.float32)
    W2 = np.asarray(inputs["W2"], np.float32)
    b2 = np.asarray(inputs["b2"], np.float32)
    lin_w = np.asarray(inputs["lin_w"], np.float32)
    lin_b = np.asarray(inputs["lin_b"], np.float32)
    C, H2 = cfg.C, cfg.H2

    plans, dis = preprocess(inputs["edge_index"], inputs["edge_logits"], cfg)
    dis2 = (dis * dis).astype(np.float32)
    runners = _get_runners(plans, cfg)

    W2p = np.zeros((C, C), np.float32)
    W2p[:, :H2] = W2
    b2p = np.zeros(C, np.float32)
    b2p[:H2] = b2
    lwp = np.zeros((C, 1), np.float32)
    lwp[:H2, 0] = lin_w[:, 0]
    lbp = lin_b.reshape(1, 1)

    x_pad = np.zeros((cfg.N, cfg.ROWB), BF16)
    x_pad[:, :C] = x.astype(BF16)

    def phase_inputs(d, table_pad, table_f32, Wd, bdv, lwv, lbv):
        p = plans[d]
        sh = slice(d * cfg.NLOC, (d + 1) * cfg.NLOC)
        sxT = np.ascontiguousarray((table_f32[sh] * dis2[sh, None]).T)
        m = dict(table=table_pad, sxT=sxT, Wd=Wd, bb=np.tile(bdv, (128, 1)),
                 bdc=bdv.reshape(C, 1), lw=lwv, lb=lbv)
        for g in range(cfg.NGRP):
            m[f"idx{g}"] = p[g]["idx"]
            m[f"S{g}"] = p[g]["S"]
        return m

    # phase A: table=x, dense=W1/b1 -> h1 (bf16, padded rows)
    with ThreadPoolExecutor(cfg.P) as exe:
        resA = list(exe.map(
            lambda d: runners[d][0](phase_inputs(
                d, x_pad, x, W1, b1, lwp, lbp)),
            range(cfg.P)))
    h1_pad = np.concatenate([np.asarray(r["h_out"]) for r in resA], axis=0)
    h1_f32 = h1_pad[:, :C].astype(np.float32)

    # phase B: table=h1, dense=padded W2/b2, head=lin
    with ThreadPoolExecutor(cfg.P) as exe:
        resB = list(exe.map(
            lambda d: runners[d][1](phase_inputs(
                d, h1_pad, h1_f32, W2p, b2p, lwp, lbp)),
            range(cfg.P)))
    out = np.concatenate([np.asarray(r["outT"]).T for r in resB], axis=0)
    return out.astype(np.float32)


def kernel(x, edge_index, edge_logits, W1, b1, W2, b2, lin_w, lin_b):
    inputs = dict(x=x, edge_index=edge_index, edge_logits=edge_logits,
                  W1=W1, b1=b1, W2=W2, b2=b2, lin_w=lin_w, lin_b=lin_b)
    return run_two_phase(inputs, FULL)


# revision 10
# speedup vs baseline: 1.0215x; 1.0215x over previous
"""Trainium2 Bass kernel for a 2-layer edge-gated GCN (DiffGNNPlacement).

Math (reference, per layer):
    ew   = 0.5 + sigmoid(edge_logits)                  # [E]
    deg  = segsum(ew -> col) + 1                       # [N]
    dis  = deg^-1/2
    norm = dis[row] * ew * dis[col]                    # [E]
    out  = segsum(norm * (h@W)[row] -> col) + (h@W)*dis^2 + b

Key transform: aggregation commutes with the (linear) feature transform, so
    out = (segsum(norm * h[row] -> col) + h*dis^2) @ W + b
and the self-loop term (h*dis^2, transposed) is added host-side via sxT.

Device algorithm (per core, nodes sharded 12500/core):
  - edges partitioned by target shard, sorted by target col, grouped by
    source chunk of 25000 rows (dma_gather indices are int16), packed into
    128-slot tiles spanning <=32 target cols within a 512-col PSUM window.
  - per tile: dma_gather 128 rows (256B each: 64 bf16 channels + pad) of the
    feature table -> SBUF; a host-built one-hot-times-norm matrix S
    [128, 32] bf16; PE matmul psum[64, off:off+w] += gathered^T @ S.
  - the 4 source groups use the 4 SWDGE queues (queue_num=g), so descriptor
    generation runs on all 8 GPSIMD Q7 cores concurrently (4 pairs) instead
    of serializing on pair 0 -- the single biggest win over one queue.
  - per window: psum + sxT window -> z window (f32), then immediately the
    dense stage for that window (phase A: h = relu(z@W1+b1) row-major, bf16,
    padded rows for the next layer's gather table; phase B: transposed
    dense + classifier head -> outT), overlapping the tail with later
    windows' gathers.

Two specialized programs per core (A: agg+dense, B: agg+head); host
all-gathers h1 between the launches.
"""

import os
import sys
import numpy as np
from contextlib import ExitStack

for _p in ("/opt/trn_rl_repo", "/root/.axon_site/_ro/trn_rl_repo"):
    if os.path.isdir(_p) and _p not in sys.path:
        sys.path.insert(0, _p)

import ml_dtypes

BF16 = np.dtype(ml_dtypes.bfloat16)


# ----------------------------------------------------------------- config ---
class Cfg:
    def __init__(self, N=100000, E=1600000, C=64, H2=32, P=8,
                 SRC_CHUNK=25000, W=32, WIN=512, TCH=32):
        self.N, self.E, self.C, self.H2, self.P = N, E, C, H2, P
        self.NLOC = N // P
        self.SRC_CHUNK = SRC_CHUNK
        self.NGRP = (N + SRC_CHUNK - 1) // SRC_CHUNK
        self.W = W            # S tile width (target-col window per tile)
        self.WIN = WIN        # PSUM accumulation window (cols)
        self.TCH = TCH        # tiles per gather chunk
        self.NWIN = (self.NLOC + WIN - 1) // WIN
        self.ROWB = 128       # padded table row: 128 bf16 elems = 256B
        assert SRC_CHUNK <= 32767
        assert self.NGRP == 4  # one SWDGE queue per source group


FULL = Cfg()


# --------------------------------------------------------- host preprocess ---
def _sigmoid(x):
    return 0.5 * (np.tanh(0.5 * x) + 1.0)


def preprocess(edge_index, edge_logits, cfg=FULL):
    """Compute norms and per-device tile plans (pure numpy)."""
    N, NLOC, G = cfg.N, cfg.NLOC, cfg.NGRP
    row = np.asarray(edge_index[0], dtype=np.int64)
    col = np.asarray(edge_index[1], dtype=np.int64)
    ew = (0.5 + _sigmoid(np.asarray(edge_logits, dtype=np.float32))).astype(np.float32)
    deg = np.bincount(col, weights=ew.astype(np.float64), minlength=N).astype(np.float32) + 1.0
    dis = deg ** -0.5
    norm = (dis[row] * ew * dis[col]).astype(np.float32)

    a_row, a_col, a_val = row, col, norm
    dev = a_col // NLOC
    grp = a_row // cfg.SRC_CHUNK
    order = np.lexsort((a_col, grp, dev))
    a_row, a_col, a_val = a_row[order], a_col[order], a_val[order]
    dev, grp = dev[order], grp[order]

    key = dev * G + grp
    bounds = np.searchsorted(key, np.arange(cfg.P * G + 1))
    plans = []
    for d in range(cfg.P):
        gplans = []
        for g in range(G):
            a, b = bounds[d * G + g], bounds[d * G + g + 1]
            gplans.append(_plan_group(
                (a_row[a:b] - g * cfg.SRC_CHUNK).astype(np.int16),
                (a_col[a:b] - d * NLOC).astype(np.int32),
                a_val[a:b], cfg))
        plans.append(gplans)
    return plans, dis


def _plan_group(rows, cols, vals, cfg):
    """Tile a sorted-by-col edge list: 128-slot tiles, <=W col span (tiles
    may straddle a WIN boundary). Returns packed gather/S arrays."""
    m = len(cols)
    starts, c0s = [], []
    i = 0
    while i < m:
        c0 = int(cols[i])
        # span-limited only; tiles may straddle a WIN boundary (the matmul
        # is split into two PSUM windows at build time)
        jmax = min(i + 128, m)
        j = i + int(np.searchsorted(cols[i:jmax], c0 + cfg.W, side="left"))
        starts.append(i)
        c0s.append(c0)
        i = j
    T = len(c0s)
    starts_a = np.array(starts + [m], dtype=np.int64)
    c0s = np.array(c0s, dtype=np.int32)

    tile_of = np.repeat(np.arange(T), np.diff(starts_a))
    slot = np.arange(m) - starts_a[tile_of]
    idx16 = np.zeros((T, 128), np.int16)
    idx16[tile_of, slot] = rows
    S = np.zeros((T, 128, cfg.W), np.float32)
    S[tile_of, slot, cols - c0s[tile_of]] = vals

    # chunk packing
    TCH = cfg.TCH
    nch = max(1, (T + TCH - 1) // TCH)
    Tp = nch * TCH
    flat = np.zeros(Tp * 128, np.int16)
    flat[: T * 128] = idx16.reshape(-1)
    # wrap: idx i -> [i % 16, i // 16], replicated across 8 groups of 16 partitions
    wrapped = flat.reshape(nch, TCH * 128 // 16, 16).transpose(0, 2, 1)  # [nch,16,TCH*8]
    idx_w = np.ascontiguousarray(np.tile(wrapped, (1, 8, 1)))            # [nch,128,TCH*8]
    Sp = np.zeros((Tp, 128, cfg.W), np.float32)
    Sp[:T] = S
    S_pk = np.ascontiguousarray(
        Sp.reshape(nch, TCH, 128, cfg.W).transpose(0, 2, 1, 3)).astype(BF16)  # [nch,128,TCH,W]
    nids = [min(TCH, T - ch * TCH) * 128 for ch in range(nch)]

    win = c0s // cfg.WIN
    off = c0s - win * cfg.WIN
    return dict(T=T, nch=nch, idx=idx_w, S=S_pk, nids=nids, win=win, off=off)


# ---------------------------------------------------------- program builder ---
def build_program(plan_d, phase, cfg=FULL, name="gnn"):
    import concourse.bass as bass
    import concourse.mybir as mybir
    from concourse import bacc
    from concourse.tile import TileContext

    f32, bf16, i16 = mybir.dt.float32, mybir.dt.bfloat16, mybir.dt.int16
    C, W, WIN, TCH, NLOC = cfg.C, cfg.W, cfg.WIN, cfg.TCH, cfg.NLOC
    G, ROWB = cfg.NGRP, cfg.ROWB

    nc = bacc.Bacc("TRN2", enable_partition_id=False,
                   target_bir_lowering=False, name=name,
                   num_swdge_queues=4)

    table = nc.dram_tensor("table", [cfg.N, ROWB], bf16, kind="ExternalInput")
    sxT_dr = nc.dram_tensor("sxT", [C, NLOC], f32, kind="ExternalInput")
    Wd = nc.dram_tensor("Wd", [C, C], f32, kind="ExternalInput")
    bb_dr = nc.dram_tensor("bb", [128, C], f32, kind="ExternalInput")
    bdc = nc.dram_tensor("bdc", [C, 1], f32, kind="ExternalInput")
    lw = nc.dram_tensor("lw", [C, 1], f32, kind="ExternalInput")
    lb = nc.dram_tensor("lb", [1, 1], f32, kind="ExternalInput")
    idx_dr, S_dr = [], []
    for g in range(G):
        p = plan_d[g]
        idx_dr.append(nc.dram_tensor(f"idx{g}", list(p["idx"].shape), i16,
                                     kind="ExternalInput"))
        S_dr.append(nc.dram_tensor(f"S{g}", list(p["S"].shape), bf16,
                                   kind="ExternalInput"))
    if phase == "A":
        h_out = nc.dram_tensor("h_out", [NLOC, ROWB], bf16, kind="ExternalOutput")
    else:
        outT = nc.dram_tensor("outT", [2, NLOC], f32, kind="ExternalOutput")

    # per-window tile lists: (g, t, off, s_lo, weff); straddling tiles get
    # a second entry in the next window with an S-column offset
    win_tiles = [[] for _ in range(cfg.NWIN)]
    for g in range(G):
        p = plan_d[g]
        for t in range(p["T"]):
            w = int(p["win"][t])
            off = int(p["off"][t])
            wlen = min(WIN, NLOC - w * WIN)
            w1 = min(W, WIN - off)
            win_tiles[w].append((g, t, off, 0, min(w1, wlen - off)))
            if W > w1 and w + 1 < cfg.NWIN:
                wlen2 = min(WIN, NLOC - (w + 1) * WIN)
                win_tiles[w + 1].append((g, t, 0, w1, min(W - w1, wlen2)))

    with TileContext(nc) as tc, ExitStack() as ex:
        cpool = ex.enter_context(tc.tile_pool(name="consts", bufs=1))
        zpool = ex.enter_context(tc.tile_pool(name="z", bufs=3))
        sxpool = ex.enter_context(tc.tile_pool(name="sx", bufs=3))
        gpools = [ex.enter_context(tc.tile_pool(name=f"gat{g}", bufs=3)) for g in range(G)]
        ipools = [ex.enter_context(tc.tile_pool(name=f"idx{g}", bufs=4)) for g in range(G)]
        spools = [ex.enter_context(tc.tile_pool(name=f"s{g}", bufs=3)) for g in range(G)]
        ppool = ex.enter_context(tc.tile_pool(name="psagg", bufs=2, space="PSUM"))
        if phase == "A":
            pdpool = ex.enter_context(tc.tile_pool(name="psd", bufs=3, space="PSUM"))
            hpool = ex.enter_context(tc.tile_pool(name="hrows", bufs=3))
        else:
            ptpool = ex.enter_context(tc.tile_pool(name="pst", bufs=2, space="PSUM"))
            plpool = ex.enter_context(tc.tile_pool(name="psl", bufs=2, space="PSUM"))
            htpool = ex.enter_context(tc.tile_pool(name="ht", bufs=2))
            opool = ex.enter_context(tc.tile_pool(name="ot", bufs=3))

        # ---- constants
        zrow = cpool.tile([1, WIN], bf16)
        nc.vector.memset(zrow[:, :], 0.0)
        Wd_sb = cpool.tile([C, C], f32)
        nc.sync.dma_start(out=Wd_sb[:, :], in_=Wd[:, :])
        if phase == "A":
            bb = cpool.tile([128, C], f32)
            nc.sync.dma_start(out=bb[:, :], in_=bb_dr[:, :])
        else:
            bd_col = cpool.tile([C, 1], f32)
            nc.sync.dma_start(out=bd_col[:, :], in_=bdc[:, :])
            lw_sb = cpool.tile([C, 1], f32)
            nc.sync.dma_start(out=lw_sb[:, :], in_=lw[:, :])
            lb_sb = cpool.tile([1, 1], f32)
            nc.sync.dma_start(out=lb_sb[:, :], in_=lb[:, :])
            nlb = cpool.tile([1, 1], f32)
            nc.scalar.mul(nlb[:, :], lb_sb[:, :], -1.0)

        # ---- aggregation state
        cur = [dict(ch=-1, gb=None, sb=None) for _ in range(G)]

        def ensure_chunk(g, ch):
            st = cur[g]
            if st["ch"] == ch:
                return st
            p = plan_d[g]
            ntl = min(TCH, p["T"] - ch * TCH)
            nid = p["nids"][ch]
            ib = ipools[g].tile([128, TCH * 8], i16, tag="idx")
            nc.sync.dma_start(out=ib[:, : ntl * 8], in_=idx_dr[g][ch, :, : ntl * 8])
            sb = spools[g].tile([128, TCH, W], bf16, tag="s")
            nc.scalar.dma_start(out=sb[:, :ntl, :], in_=S_dr[g][ch, :, :ntl, :])
            gb = gpools[g].tile([128, TCH, ROWB], bf16, tag="g")
            nc.gpsimd.dma_gather(
                gb[:, :ntl, :],
                table[g * cfg.SRC_CHUNK:(g + 1) * cfg.SRC_CHUNK, :],
                ib[:, : ntl * 8],
                nid, nid, ROWB,
                single_packet=False,
                queue_num=g,
            )
            st.update(ch=ch, gb=gb, sb=sb)
            return st

        # prefetch the first chunks so gathers start before the consts DMAs
        for g in range(G):
            ensure_chunk(g, 0)

        for w in range(cfg.NWIN):
            wlen = min(WIN, NLOC - w * WIN)
            ps = ppool.tile([C, WIN], f32)
            nc.tensor.matmul(ps[:, :wlen], lhsT=zrow[:, :C], rhs=zrow[:, :wlen],
                             start=True, stop=False)
            for g, t, off, s_lo, weff in win_tiles[w]:
                st = ensure_chunk(g, t // TCH)
                tp = t % TCH
                nc.tensor.matmul(
                    ps[:, off:off + weff],
                    lhsT=st["gb"][:, tp, :C],
                    rhs=st["sb"][:, tp, s_lo:s_lo + weff],
                    start=False, stop=False,
                    skip_group_check=True,
                )
            nc.tensor.matmul(ps[:, :wlen], lhsT=zrow[:, :C], rhs=zrow[:, :wlen],
                             start=False, stop=True)
            sxw = sxpool.tile([C, WIN], f32, tag="sx")
            nc.sync.dma_start(out=sxw[:, :wlen],
                              in_=sxT_dr[:, w * WIN:w * WIN + wlen])
            zw = zpool.tile([C, WIN], f32, tag="z")
            nc.vector.tensor_tensor(out=zw[:, :wlen], in0=ps[:, :wlen],
                                    in1=sxw[:, :wlen], op=mybir.AluOpType.add)

            if phase == "A":
                # dense, row-major (next layer's gather table), this window
                r0 = w * WIN
                nck = (wlen + 127) // 128
                hb = hpool.tile([128, (WIN + 127) // 128, C], bf16, tag="h")
                for kk in range(nck):
                    mrow = min(128, wlen - kk * 128)
                    psd = pdpool.tile([128, C], f32)
                    nc.tensor.matmul(psd[:mrow, :],
                                     lhsT=zw[:, kk * 128:kk * 128 + mrow],
                                     rhs=Wd_sb[:, :], start=True, stop=True)
                    nc.vector.tensor_tensor(out=hb[:mrow, kk, :], in0=psd[:mrow, :],
                                            in1=bb[:mrow, :], op=mybir.AluOpType.add)
                    nc.scalar.activation(hb[:mrow, kk, :], hb[:mrow, kk, :],
                                         mybir.ActivationFunctionType.Relu)
                nfull = wlen // 128
                if nfull:
                    dst = h_out[r0:r0 + nfull * 128, :C].rearrange(
                        "(t p) c -> p t c", p=128)
                    nc.sync.dma_start(out=dst, in_=hb[:, :nfull, :])
                rem = wlen - nfull * 128
                if rem:
                    nc.sync.dma_start(out=h_out[r0 + nfull * 128:r0 + wlen, :C],
                                      in_=hb[:rem, nfull, :])
            else:
                # transposed dense + classifier head, this window
                pst = ptpool.tile([C, WIN], f32)
                nc.tensor.matmul(pst[:, :wlen], lhsT=Wd_sb[:, :],
                                 rhs=zw[:, :wlen], start=True, stop=True)
                ht = htpool.tile([C, WIN], f32, tag="ht")
                nc.scalar.activation(ht[:, :wlen], pst[:, :wlen],
                                     mybir.ActivationFunctionType.Relu,
                                     bias=bd_col[:, :])
                psl = plpool.tile([1, WIN], f32)
                nc.tensor.matmul(psl[:, :wlen], lhsT=lw_sb[:, :], rhs=ht[:, :wlen],
                                 start=True, stop=True)
                otn = opool.tile([1, WIN], f32, tag="otn")
                otp = opool.tile([1, WIN], f32, tag="otp")
                nc.scalar.activation(otn[:, :wlen], psl[:, :wlen],
                                     mybir.ActivationFunctionType.Identity,
                                     bias=nlb[:, :], scale=-1.0)
                nc.scalar.activation(otp[:, :wlen], psl[:, :wlen],
                                     mybir.ActivationFunctionType.Identity,
                                     bias=lb_sb[:, :], scale=1.0)
                nc.sync.dma_start(out=outT[0:1, w * WIN:w * WIN + wlen],
                                  in_=otn[:, :wlen])
                nc.sync.dma_start(out=outT[1:2, w * WIN:w * WIN + wlen],
                                  in_=otp[:, :wlen])

    nc.compile()
    return nc


# ------------------------------------------------------------------ runner ---
def make_runner(nc, device):
    """Single-core jit runner pinned to one device, reusable across calls."""
    import jax
    import concourse.mybir as mybir
    from concourse import bass2jax

    bass2jax.install_neuronx_cc_hook()

    in_names, out_names, out_avals, zero_shapes = [], [], [], []
    for alloc in nc.m.functions[0].allocations:
        if not isinstance(alloc, mybir.MemoryLocationSet):
            continue
        nm = alloc.memorylocations[0].name
        if alloc.kind == "ExternalInput":
            in_names.append(nm)
        elif alloc.kind == "ExternalOutput":
            shape = tuple(alloc.tensor_shape)
            dtype = mybir.dt.np(alloc.dtype)
            out_names.append(nm)
            out_avals.append(jax.core.ShapedArray(shape, dtype))
            zero_shapes.append((shape, dtype))
    n_params = len(in_names)
    all_in_names = in_names + out_names
    donate = tuple(range(n_params, n_params + len(out_names)))

    def _body(*args):
        outs = bass2jax._bass_exec_p.bind(
            *args,
            out_avals=tuple(out_avals),
            in_names=tuple(all_in_names),
            out_names=tuple(out_names),
            lowering_input_output_aliases=(),
            sim_require_finite=True,
            sim_require_nnan=True,
            nc=nc,
        )
        return tuple(outs)

    jitted = jax.jit(_body, donate_argnums=donate, keep_unused=True)

    def run(in_map):
        args = [jax.device_put(np.asarray(in_map[nm]), device) for nm in in_names]
        zeros = [jax.device_put(np.zeros(s, d), device) for s, d in zero_shapes]
        outs = jitted(*args, *zeros)
        return {nm: outs[i] for i, nm in enumerate(out_names)}

    return run


# ---------------------------------------------------------------- kernel() ---
_CACHE = {}


def _get_runners(plans, cfg):
    import jax
    from concurrent.futures import ThreadPoolExecutor
    key = "runners"
    if key in _CACHE:
        return _CACHE[key]
    devices = jax.devices()[:cfg.P]

    def build_pair(d):
        ncA = build_program(plans[d], "A", cfg, name=f"gnnA_d{d}")
        ncB = build_program(plans[d], "B", cfg, name=f"gnnB_d{d}")
        return (make_runner(ncA, devices[d]), make_runner(ncB, devices[d]))

    with ThreadPoolExecutor(4) as exe:
        runners = list(exe.map(build_pair, range(cfg.P)))
    _CACHE[key] = runners
    return runners


def run_two_phase(inputs, cfg=FULL):
    from concurrent.futures import ThreadPoolExecutor

    x = np.asarray(inputs["x"], np.float32)
    W1 = np.asarray(inputs["W1"], np.float32)
    b1 = np.asarray(inputs["b1"], np.float32)
    W2 = np.asarray(inputs["W2"], np.float32)
    b2 = np.asarray(inputs["b2"], np.float32)
    lin_w = np.asarray(inputs["lin_w"], np.float32)
    lin_b = np.asarray(inputs["lin_b"], np.float32)
    C, H2 = cfg.C, cfg.H2

    plans, dis = preprocess(inputs["edge_index"], inputs["edge_logits"], cfg)
    dis2 = (dis * dis).astype(np.float32)
    runners = _get_runners(plans, cfg)

    W2p = np.zeros((C, C), np.float32)
    W2p[:, :H2] = W2
    b2p = np.zeros(C, np.float32)
    b2p[:H2] = b2
    lwp = np.zeros((C, 1), np.float32)
    lwp[:H2, 0] = lin_w[:, 0]
    lbp = lin_b.reshape(1, 1)

    x_pad = np.zeros((cfg.N, cfg.ROWB), BF16)
    x_pad[:, :C] = x.astype(BF16)

    def phase_inputs(d, table_pad, table_f32, Wd, bdv, lwv, lbv):
        p = plans[d]
        sh = slice(d * cfg.NLOC, (d + 1) * cfg.NLOC)
        sxT = np.ascontiguousarray((table_f32[sh] * dis2[sh, None]).T)
        m = dict(table=table_pad, sxT=sxT, Wd=Wd, bb=np.tile(bdv, (128, 1)),
                 bdc=bdv.reshape(C, 1), lw=lwv, lb=lbv)
        for g in range(cfg.NGRP):
            m[f"idx{g}"] = p[g]["idx"]
            m[f"S{g}"] = p[g]["S"]
        return m

    # phase A: table=x, dense=W1/b1 -> h1 (bf16, padded rows)
    with ThreadPoolExecutor(cfg.P) as exe:
        resA = list(exe.map(
            lambda d: runners[d][0](phase_inputs(
                d, x_pad, x, W1, b1, lwp, lbp)),
            range(cfg.P)))
    h1_pad = np.concatenate([np.asarray(r["h_out"]) for r in resA], axis=0)
    h1_f32 = h1_pad[:, :C].astype(np.float32)

    # phase B: table=h1, dense=padded W2/b2, head=lin
    with ThreadPoolExecutor(cfg.P) as exe:
        resB = list(exe.map(
            lambda d: runners[d][1](phase_inputs(
                d, h1_pad, h1_f32, W2p, b2p, lwp, lbp)),
            range(cfg.P)))
    out = np.concatenate([np.asarray(r["outT"]).T for r in resB], axis=0)
    return out.astype(np.float32)


def kernel(x, edge_index, edge_logits, W1, b1, W2, b2, lin_w, lin_b):
    inputs = dict(x=x, edge_index=edge_index, edge_logits=edge_logits,
                  W1=W1, b1=b1, W2=W2, b2=b2, lin_w=lin_w, lin_b=lin_b)
    return run_two_phase(inputs, FULL)
